# revision 19
# baseline (speedup 1.0000x reference)
"""Multi-head causal self-attention (B=2, T=4096, C=768, H=12, D=64) on 8 NeuronCores.

Sharding: core c handles batch b = c // 4 and a group of 3 heads (c % 4).
Each core runs a fused flash-attention pipeline per 512-column tq chunk:
QKV projection -> V transpose -> streaming softmax(QK^T)V -> output projection,
producing a partial (pre-bias) out.T [768, 4096]. The host sums the 4 partials
per batch and adds the projection bias.

v2 layout: the P@V matmul is flipped to produce O natural [q, d] (moving dim =
d+1 = 65 instead of 512), which makes the softmax denominator a per-partition
scalar: reciprocal [128,1] + per-partition multiply, no cross-partition
broadcast. O is then transposed back via PE for the output projection, whose
per-head weights are stacked 2+1 on the contraction dim. exp runs without
max-subtraction (scores are O(+-6)); the denominator comes free from a
ones-column appended to V.
"""

from contextlib import ExitStack

import numpy as np

import concourse.bass as bass
import concourse.tile as tile
from concourse import bacc
from concourse import mybir
from concourse._compat import with_exitstack
from concourse.bass_utils import run_bass_kernel_spmd

F32 = mybir.dt.float32
F32R = mybir.dt.float32r
BF16 = mybir.dt.bfloat16
EXP = mybir.ActivationFunctionType.Exp

B, T, C = 2, 4096, 768
H, D = 12, 64
NCORES = 8
HPC = 3           # heads per core
GPB = NCORES // B  # head-group cores per batch (4)
TQ = 512          # tq chunk width
NJ = T // TQ      # 8
TKB = 128         # tk block
NB = T // TKB     # 32
KC = C // 128     # 6 contraction chunks for the QKV projection
SCALE = 1.0 / np.sqrt(D)

# Layout of the per-core QKV weight columns: 5 chunks of 128 (last half-used).
# Each entry is (quantity, local head, partition base within the chunk).
# Chosen so Q and K of the same head land on the same partition half (their
# zero-padded halves line up in the 128-deep score contraction).
CHUNKS = [
    [("Q", 0, 0), ("Q", 1, 64)],
    [("Q", 2, 0), ("V", 0, 64)],
    [("K", 0, 0), ("K", 1, 64)],
    [("K", 2, 0), ("V", 1, 64)],
    [("V", 2, 0)],
]
NQKV = 4 * 128 + 64  # 576 columns of per-core qkv weights

QK_BASE = {0: 0, 1: 64, 2: 0}   # partition base of Q/K data per local head

DEBUG = False  # add j=0 intermediate dumps for bisection


def _proj(nc, ps_p, stp, wp01_sb, wp2_sb, outT_r, ot01, ot2, j):
    jsl = slice(j * TQ, (j + 1) * TQ)
    for m in range(KC):
        ps3 = ps_p.tile([128, TQ], F32, tag="misc", name="ps3")
        nc.tensor.matmul(
            ps3[:],
            lhsT=wp01_sb[:, m * 128:(m + 1) * 128],
            rhs=ot01[:],
            start=True,
            stop=False,
        )
        nc.tensor.matmul(
            ps3[:],
            lhsT=wp2_sb[:, m * 128:(m + 1) * 128],
            rhs=ot2[:],
            start=False,
            stop=True,
        )
        st = stp.tile([128, TQ], F32, tag="st", name="st")
        nc.vector.tensor_copy(st[:], ps3[:])
        nc.sync.dma_start(outT_r[:, m, jsl], st[:])


@with_exitstack
def _mhsa_body(ctx: ExitStack, tc: tile.TileContext, t):
    nc = tc.nc
    xT_r = t["xT"].rearrange("(kc p) t -> p kc t", p=128)
    outT_r = t["outT"].rearrange("(mo p) t -> p mo t", p=128)

    const = ctx.enter_context(tc.tile_pool(name="const", bufs=1))
    persist = ctx.enter_context(tc.tile_pool(name="persist", bufs=1))
    xpool = ctx.enter_context(tc.tile_pool(name="xpool", bufs=2))
    vstp = ctx.enter_context(tc.tile_pool(name="vstp", bufs=2))
    ptp = ctx.enter_context(tc.tile_pool(name="ptp", bufs=18))
    onp = ctx.enter_context(tc.tile_pool(name="onp", bufs=2))
    rcp = ctx.enter_context(tc.tile_pool(name="rcp", bufs=4))
    otp = ctx.enter_context(tc.tile_pool(name="otp", bufs=2))
    stp = ctx.enter_context(tc.tile_pool(name="stp", bufs=3))

    ps_s = ctx.enter_context(tc.tile_pool(name="ps_s", bufs=2, space="PSUM"))
    ps_o = ctx.enter_context(tc.tile_pool(name="ps_o", bufs=2, space="PSUM"))
    ps_misc = ctx.enter_context(tc.tile_pool(name="ps_misc", bufs=2, space="PSUM"))

    wq_sb = const.tile([128, KC, NQKV], F32R)
    nc.sync.dma_start(wq_sb[:], t["wqkv"].rearrange("(kc p) m -> p kc m", p=128))
    bias_sb = const.tile([128, 5], F32)
    nc.sync.dma_start(bias_sb[:], t["bqkv"].rearrange("m p -> p m"))
    wp01_sb = const.tile([128, C], BF16)
    nc.sync.dma_start(wp01_sb[:], t["wproj01"])
    wp2_sb = const.tile([64, C], BF16)
    nc.sync.dma_start(wp2_sb[:], t["wproj2"])
    id_sb = const.tile([128, 128], BF16)
    nc.sync.dma_start(id_sb[:], t["ident"])
    mask_sb = const.tile([128, 1280], BF16)
    nc.sync.dma_start(mask_sb[:], t["masks"])

    KT = [persist.tile([128, T], BF16, tag=f"KT{h}", name=f"KT{h}") for h in range(HPC)]
    Vp = [
        persist.tile([128, NB, D + 1], BF16, tag=f"Vp{h}", name=f"Vp{h}")
        for h in range(HPC)
    ]
    # persistent double-buffered qt: pads zeroed once, never rewritten
    qt2 = persist.tile([128, 2, HPC, TQ], BF16, tag="qt2", name="qt2")

    for h in range(HPC):
        pad_lo = 64 - QK_BASE[h]  # 64 if data at 0, 0 if data at 64
        nc.vector.memset(KT[h][pad_lo:pad_lo + 64, :], 0.0)
        nc.vector.memset(Vp[h][:, :, D:D + 1], 1.0)
        for db in range(2):
            nc.vector.memset(qt2[pad_lo:pad_lo + 64, db, h, :], 0.0)

    for j in range(NJ):
        jsl = slice(j * TQ, (j + 1) * TQ)
        qt = qt2[:, j % 2]

        # ---- QKV projection for this tq chunk ----
        xt = xpool.tile([128, KC, TQ], F32R, tag="xt")
        nc.sync.dma_start(xt[:], xT_r[:, :, jsl])
        vst = {}
        for m in range(5):
            ents = CHUNKS[m]
            mw = 128 if len(ents) == 2 else 64
            ps = ps_misc.tile([128, TQ], F32, tag="misc", name="psq")
            for kc in range(KC):
                nc.tensor.matmul(
                    ps[:mw],
                    lhsT=wq_sb[:, kc, m * 128:m * 128 + mw],
                    rhs=xt[:, kc, :],
                    start=(kc == 0),
                    stop=(kc == KC - 1),
                )
            for (qty, h, base) in ents:
                if qty == "V":
                    vt = vstp.tile([64, TQ], BF16, tag=f"vst{h}")
                    vst[h] = vt
                    dst = vt[:]
                elif qty == "K":
                    dst = KT[h][base:base + 64, jsl]
                else:
                    dst = qt[base:base + 64, h, :]
                nc.vector.tensor_scalar_add(
                    dst, ps[base:base + 64, :], bias_sb[base:base + 64, m:m + 1]
                )

        # ---- V transposes: V.T [64, TQ] stage -> natural V in Vp ----
        for h in range(HPC):
            vt = vst[h]
            for s in range(4):
                pst = ps_misc.tile([128, TQ], F32, tag="misc", name="pst")
                pstv = pst.bitcast(BF16)[:, 0:64]
                nc.tensor.transpose(
                    pstv, vt[:, s * 128:(s + 1) * 128], id_sb[0:64, 0:64]
                )
                nc.vector.tensor_copy(
                    out=Vp[h][:, 4 * j + s, 0:D],
                    in_=pstv,
                )

        if DEBUG and j == 0:
            nc.sync.dma_start(t["dbg_qt"], qt2[:, 0].rearrange("p h q -> p (h q)"))
            nc.sync.dma_start(t["dbg_kt"], KT[0][:, 0:TQ])
            nc.sync.dma_start(
                t["dbg_vp"], Vp[0][:, 0:4, :].rearrange("p b d -> p (b d)")
            )

        # ---- streaming attention for this tq chunk ----
        nblk = 4 * j + 4
        npair = nblk // 2
        for h in range(HPC):
            # scores + exp for all key-block pairs of this (j, h)
            pts, cols = [], {}
            for ip in range(npair):
                i0, i1 = 2 * ip, 2 * ip + 1
                # column trim offsets: block i only contributes to tq columns
                # >= 128*(i-4j) within this chunk (the rest is fully masked)
                offs = [max(0, 128 * (i - 4 * j)) for i in (i0, i1)]
                ns = [TQ - o for o in offs]
                starts = [0, ns[0]]
                w = ns[0] + ns[1]
                pss = ps_s.tile([128, 2 * TQ], F32, tag="pss")
                for n, i in enumerate((i0, i1)):
                    cols[i] = (ip, starts[n] - offs[n])
                    nc.tensor.matmul(
                        pss[:, starts[n]:starts[n] + ns[n]],
                        lhsT=KT[h][:, i * 128:(i + 1) * 128],
                        rhs=qt[:, h, offs[n]:TQ],
                        start=True,
                        stop=True,
                    )
                pt = ptp.tile([128, 2 * TQ], BF16, tag="pt")
                pts.append(pt)
                nc.scalar.activation(
                    out=pt[:, :w], in_=pss[:, :w], func=EXP, scale=SCALE
                )
                rp = ip - 2 * j
                if rp >= 0:  # diagonal pair: packed masks for both blocks
                    moff = 0 if rp == 0 else 896
                    nc.vector.tensor_mul(
                        pt[:, :w], pt[:, :w], mask_sb[:, moff:moff + w]
                    )
                if DEBUG and j == 0 and h == 0 and ip == 0:
                    nc.sync.dma_start(t["dbg_pt"], pt[:, 0:896])
            # flipped P@V, one accumulation group (= one PSUM bank) per
            # 128-query block qb: stationary = P^T slice, moving = V
            rc = rcp.tile([128, 4], F32, tag="rc")
            on = onp.tile([128, 4, D], BF16, tag="on")
            for qb in range(4):
                pso = ps_o.tile([128, 512], F32, tag="pso")
                last = 4 * j + qb
                for i in range(last + 1):
                    ip, base = cols[i]
                    nc.tensor.matmul(
                        pso[:, 0:D + 1],
                        lhsT=pts[ip][:, base + qb * 128:base + qb * 128 + 128],
                        rhs=Vp[h][:, i, :],
                        start=(i == 0),
                        stop=(i == last),
                    )
                # normalize: O rows / per-partition denominator (column D)
                nc.vector.reciprocal(rc[:, qb:qb + 1], pso[:, D:D + 1])
                nc.vector.tensor_scalar_mul(
                    on[:, qb, :], pso[:, 0:D], rc[:, qb:qb + 1]
                )
            if DEBUG and j == 0 and h == 0:
                nc.sync.dma_start(t["dbg_rc"], rc[:])
            # transpose O back to [d, q] for the projection
            ptTt = ps_misc.tile([128, TQ], F32, tag="misc", name="ptT")
            ptT = ptTt.bitcast(BF16)[0:64, 0:512].rearrange(
                "p (qb c) -> p qb c", c=128
            )
            for qb in range(4):
                nc.tensor.transpose(ptT[:, qb, :], on[:, qb, :], id_sb[:])
            if DEBUG and j == 0 and h == 0:
                nc.sync.dma_start(
                    t["dbg_on"], on[:].rearrange("p b d -> p (b d)")
                )
            if h < 2:
                if h == 0:
                    ot01 = otp.tile([128, 4, 128], BF16, tag="ot01")
                nc.vector.tensor_copy(out=ot01[h * 64:(h + 1) * 64], in_=ptT[:])
            else:
                ot2 = otp.tile([64, 4, 128], BF16, tag="ot2")
                nc.vector.tensor_copy(out=ot2[:], in_=ptT[:])
        if DEBUG and j == 0:
            nc.sync.dma_start(
                t["dbg_ot"], ot01[:].rearrange("p b d -> p (b d)")
            )

        # ---- output projection, software-pipelined by one chunk ----
        if j > 0:
            _proj(nc, ps_misc, stp, wp01_sb, wp2_sb, outT_r, prev01, prev2, j - 1)
        prev01, prev2 = ot01, ot2
    _proj(nc, ps_misc, stp, wp01_sb, wp2_sb, outT_r, prev01, prev2, NJ - 1)


def build_nc():
    nc = bacc.Bacc("TRN2", target_bir_lowering=False, debug=False)
    t = {}
    t["xT"] = nc.dram_tensor("xT", [C, T], F32R, kind="ExternalInput").ap()
    t["wqkv"] = nc.dram_tensor("wqkv", [C, NQKV], F32R, kind="ExternalInput").ap()
    t["bqkv"] = nc.dram_tensor("bqkv", [5, 128], F32, kind="ExternalInput").ap()
    t["wproj01"] = nc.dram_tensor("wproj01", [128, C], BF16, kind="ExternalInput").ap()
    t["wproj2"] = nc.dram_tensor("wproj2", [64, C], BF16, kind="ExternalInput").ap()
    t["ident"] = nc.dram_tensor("ident", [128, 128], BF16, kind="ExternalInput").ap()
    t["masks"] = nc.dram_tensor("masks", [128, 1280], BF16, kind="ExternalInput").ap()
    t["outT"] = nc.dram_tensor("outT", [C, T], F32, kind="ExternalOutput").ap()
    if DEBUG:
        for nm, shp, dt in [
            ("dbg_qt", [128, HPC * TQ], BF16),
            ("dbg_kt", [128, TQ], BF16),
            ("dbg_vp", [128, 4 * (D + 1)], BF16),
            ("dbg_pt", [128, 896], BF16),
            ("dbg_on", [128, 4 * D], BF16),
            ("dbg_rc", [128, 4], F32),
            ("dbg_ot", [128, 4 * 128], BF16),
        ]:
            t[nm] = nc.dram_tensor(nm, shp, dt, kind="ExternalOutput").ap()
    with tile.TileContext(nc) as tc:
        _mhsa_body(tc, t)
    nc.compile()
    return nc


def make_in_maps(x, W_qkv, b_qkv, W_proj):
    """Shard the full inputs into one input map per core."""
    import ml_dtypes

    x = np.asarray(x, dtype=np.float32)
    W_qkv = np.asarray(W_qkv, dtype=np.float32)
    b_qkv = np.asarray(b_qkv, dtype=np.float32)
    W_proj = np.asarray(W_proj, dtype=np.float32)

    ident = np.eye(128, dtype=ml_dtypes.bfloat16)
    q_idx = np.arange(TQ)
    p_idx = np.arange(128)
    m4 = np.zeros((4, 128, TQ), dtype=np.float32)
    for r in range(4):
        m4[r] = (p_idx[:, None] <= (q_idx[None, :] - 128 * r)).astype(np.float32)
    masks = np.concatenate(
        [m4[0], m4[1][:, 128:], m4[2][:, 256:], m4[3][:, 384:]], axis=1
    ).astype(ml_dtypes.bfloat16)  # [128, 512+384+256+128 = 1280]

    in_maps = []
    for c in range(NCORES):
        b = c // GPB
        g = c % GPB
        heads = [HPC * g + h for h in range(HPC)]

        wg = np.zeros((C, NQKV), dtype=np.float32)
        bg = np.zeros((5, 128), dtype=np.float32)
        qty_off = {"Q": 0, "K": C, "V": 2 * C}
        for m, ents in enumerate(CHUNKS):
            for (qty, h, base) in ents:
                src = qty_off[qty] + heads[h] * D
                wg[:, m * 128 + base:m * 128 + base + D] = W_qkv[:, src:src + D]
                bg[m, base:base + D] = b_qkv[src:src + D]

        wp01 = np.concatenate(
            [
                W_proj[heads[0] * D:(heads[0] + 1) * D, :],
                W_proj[heads[1] * D:(heads[1] + 1) * D, :],
            ],
            axis=0,
        ).astype(ml_dtypes.bfloat16)
        wp2 = W_proj[heads[2] * D:(heads[2] + 1) * D, :].astype(ml_dtypes.bfloat16)

        in_maps.append({
            "xT": np.ascontiguousarray(x[b].T),
            "wqkv": wg,
            "bqkv": bg,
            "wproj01": wp01,
            "wproj2": wp2,
            "ident": ident,
            "masks": masks,
        })
    return in_maps


def run_cores(inputs, trace=False, **kw):
    nc = build_nc()
    in_maps = make_in_maps(
        inputs["x"], inputs["W_qkv"], inputs["b_qkv"], inputs["W_proj"]
    )
    res = run_bass_kernel_spmd(nc, in_maps, list(range(NCORES)), trace=trace, **kw)
    return res


def gather(results, b_proj):
    out = np.zeros((B, T, C), dtype=np.float32)
    for c in range(NCORES):
        out[c // GPB] += results[c]["outT"].T
    out += np.asarray(b_proj, dtype=np.float32)
    return out


def kernel(x, W_qkv, b_qkv, W_proj, b_proj):
    res = run_cores(
        {"x": x, "W_qkv": W_qkv, "b_qkv": b_qkv, "W_proj": W_proj}
    )
    return gather(res.results, b_proj)


# revision 20
# speedup vs baseline: 1.2080x; 1.2080x over previous
"""Multi-head causal self-attention (B=2, T=4096, C=768, H=12, D=64) on 8 NeuronCores.

Sharding: core c handles batch b = c // 4 and a group of 3 heads (c % 4).
Each core runs a fused flash-attention pipeline per 512-column tq chunk:
QKV projection -> V transpose -> streaming softmax(QK^T)V -> output projection,
producing a partial (pre-bias) out.T [768, 4096]. The host sums the 4 partials
per batch and adds the projection bias.

v2 layout: the P@V matmul is flipped to produce O natural [q, d] (moving dim =
d+1 = 65 instead of 512), which makes the softmax denominator a per-partition
scalar: reciprocal [128,1] + per-partition multiply, no cross-partition
broadcast. O is then transposed back via PE for the output projection, whose
per-head weights are stacked 2+1 on the contraction dim. exp runs without
max-subtraction (scores are O(+-6)); the denominator comes free from a
ones-column appended to V.
"""

from contextlib import ExitStack

import numpy as np

import concourse.bass as bass
import concourse.tile as tile
from concourse import bacc
from concourse import mybir
from concourse._compat import with_exitstack
from concourse.bass_utils import run_bass_kernel_spmd

F32 = mybir.dt.float32
F32R = mybir.dt.float32r
BF16 = mybir.dt.bfloat16
EXP = mybir.ActivationFunctionType.Exp

B, T, C = 2, 4096, 768
H, D = 12, 64
NCORES = 8
HPC = 3           # heads per core
GPB = NCORES // B  # head-group cores per batch (4)
TQ = 512          # tq chunk width
NJ = T // TQ      # 8
TKB = 128         # tk block
NB = T // TKB     # 32
KC = C // 128     # 6 contraction chunks for the QKV projection
SCALE = 1.0 / np.sqrt(D)

# Layout of the per-core QKV weight columns: 5 chunks of 128 (last half-used).
# Each entry is (quantity, local head, partition base within the chunk).
# Chosen so Q and K of the same head land on the same partition half (their
# zero-padded halves line up in the 128-deep score contraction).
CHUNKS = [
    [("Q", 0, 0), ("Q", 1, 64)],
    [("Q", 2, 0), ("V", 0, 64)],
    [("K", 0, 0), ("K", 1, 64)],
    [("K", 2, 0), ("V", 1, 64)],
    [("V", 2, 0)],
]
NQKV = 4 * 128 + 64  # 576 columns of per-core qkv weights

QK_BASE = {0: 0, 1: 64, 2: 0}   # partition base of Q/K data per local head

DEBUG = False  # add j=0 intermediate dumps for bisection


def _proj(nc, ps_p, stp, wp01_sb, wp2_sb, outT_r, ot01, ot2, j):
    jsl = slice(j * TQ, (j + 1) * TQ)
    for m in range(KC):
        ps3 = ps_p.tile([128, TQ], F32, tag="misc", name="ps3")
        nc.tensor.matmul(
            ps3[:],
            lhsT=wp01_sb[:, m * 128:(m + 1) * 128],
            rhs=ot01[:],
            start=True,
            stop=False,
        )
        nc.tensor.matmul(
            ps3[:],
            lhsT=wp2_sb[:, m * 128:(m + 1) * 128],
            rhs=ot2[:],
            start=False,
            stop=True,
        )
        st = stp.tile([128, TQ], F32, tag="st", name="st")
        nc.vector.tensor_copy(st[:], ps3[:])
        nc.sync.dma_start(outT_r[:, m, jsl], st[:])


@with_exitstack
def _mhsa_body(ctx: ExitStack, tc: tile.TileContext, t):
    nc = tc.nc
    xT_r = t["xT"].rearrange("(kc p) t -> p kc t", p=128)
    outT_r = t["outT"].rearrange("(mo p) t -> p mo t", p=128)

    const = ctx.enter_context(tc.tile_pool(name="const", bufs=1))
    persist = ctx.enter_context(tc.tile_pool(name="persist", bufs=1))
    xpool = ctx.enter_context(tc.tile_pool(name="xpool", bufs=2))
    vstp = ctx.enter_context(tc.tile_pool(name="vstp", bufs=2))
    ptp = ctx.enter_context(tc.tile_pool(name="ptp", bufs=18))
    onp = ctx.enter_context(tc.tile_pool(name="onp", bufs=2))
    rcp = ctx.enter_context(tc.tile_pool(name="rcp", bufs=4))
    otp = ctx.enter_context(tc.tile_pool(name="otp", bufs=2))
    stp = ctx.enter_context(tc.tile_pool(name="stp", bufs=3))

    ps_s = ctx.enter_context(tc.tile_pool(name="ps_s", bufs=2, space="PSUM"))
    ps_o = ctx.enter_context(tc.tile_pool(name="ps_o", bufs=2, space="PSUM"))
    ps_misc = ctx.enter_context(tc.tile_pool(name="ps_misc", bufs=2, space="PSUM"))

    wq_sb = const.tile([128, KC, NQKV], BF16)
    nc.sync.dma_start(wq_sb[:], t["wqkv"].rearrange("(kc p) m -> p kc m", p=128))
    bias_sb = const.tile([128, 5], F32)
    nc.sync.dma_start(bias_sb[:], t["bqkv"].rearrange("m p -> p m"))
    wp01_sb = const.tile([128, C], BF16)
    nc.sync.dma_start(wp01_sb[:], t["wproj01"])
    wp2_sb = const.tile([64, C], BF16)
    nc.sync.dma_start(wp2_sb[:], t["wproj2"])
    id_sb = const.tile([128, 128], BF16)
    nc.sync.dma_start(id_sb[:], t["ident"])
    mask_sb = const.tile([128, 1280], BF16)
    nc.sync.dma_start(mask_sb[:], t["masks"])

    KT = [persist.tile([128, T], BF16, tag=f"KT{h}", name=f"KT{h}") for h in range(HPC)]
    Vp = [
        persist.tile([128, NB, D + 1], BF16, tag=f"Vp{h}", name=f"Vp{h}")
        for h in range(HPC)
    ]
    # persistent double-buffered qt: pads zeroed once, never rewritten
    qt2 = persist.tile([128, 2, HPC, TQ], BF16, tag="qt2", name="qt2")

    for h in range(HPC):
        pad_lo = 64 - QK_BASE[h]  # 64 if data at 0, 0 if data at 64
        nc.vector.memset(KT[h][pad_lo:pad_lo + 64, :], 0.0)
        nc.vector.memset(Vp[h][:, :, D:D + 1], 1.0)
        for db in range(2):
            nc.vector.memset(qt2[pad_lo:pad_lo + 64, db, h, :], 0.0)

    def qkv_pieces(j):
        """Closures emitting QKV slabs + V transposes for chunk j; xt DMA now."""
        jsl = slice(j * TQ, (j + 1) * TQ)
        qt = qt2[:, j % 2]
        xt = xpool.tile([128, KC, TQ], BF16, tag="xt")
        nc.sync.dma_start(xt[:], xT_r[:, :, jsl])
        vst = {}
        pieces = []

        def slab(m, ents, mw):
            ps = ps_misc.tile([128, TQ], F32, tag="misc", name="psq")
            for kc in range(KC):
                nc.tensor.matmul(
                    ps[:mw],
                    lhsT=wq_sb[:, kc, m * 128:m * 128 + mw],
                    rhs=xt[:, kc, :],
                    start=(kc == 0),
                    stop=(kc == KC - 1),
                )
            for (qty, h, base) in ents:
                if qty == "V":
                    vt = vstp.tile([64, TQ], BF16, tag=f"vst{h}")
                    vst[h] = vt
                    dst = vt[:]
                elif qty == "K":
                    dst = KT[h][base:base + 64, jsl]
                else:
                    dst = qt[base:base + 64, h, :]
                nc.vector.tensor_scalar_add(
                    dst, ps[base:base + 64, :], bias_sb[base:base + 64, m:m + 1]
                )

        for m in range(5):
            ents = CHUNKS[m]
            mw = 128 if len(ents) == 2 else 64
            pieces.append(lambda m=m, ents=ents, mw=mw: slab(m, ents, mw))

        def vtr(h, sl):
            vt = vst[h]
            pst = ps_misc.tile([128, TQ], F32, tag="misc", name="pst")
            pstv = pst.bitcast(BF16)[:, 0:64]
            nc.tensor.transpose(
                pstv, vt[:, sl * 128:(sl + 1) * 128], id_sb[0:64, 0:64]
            )
            nc.vector.tensor_copy(out=Vp[h][:, 4 * j + sl, 0:D], in_=pstv)

        for h in range(HPC):
            for sl in range(4):
                pieces.append(lambda h=h, sl=sl: vtr(h, sl))
        return pieces

    def proj_pieces(j, ot01, ot2):
        pieces = []

        def pm(m):
            jsl = slice(j * TQ, (j + 1) * TQ)
            ps3 = ps_misc.tile([128, TQ], F32, tag="misc", name="ps3")
            nc.tensor.matmul(
                ps3[:], lhsT=wp01_sb[:, m * 128:(m + 1) * 128], rhs=ot01[:],
                start=True, stop=False,
            )
            nc.tensor.matmul(
                ps3[:], lhsT=wp2_sb[:, m * 128:(m + 1) * 128], rhs=ot2[:],
                start=False, stop=True,
            )
            st = stp.tile([128, TQ], F32, tag="st", name="st")
            nc.vector.tensor_copy(st[:], ps3[:])
            nc.sync.dma_start(outT_r[:, m, jsl], st[:])

        for m in range(KC):
            pieces.append(lambda m=m: pm(m))
        return pieces

    from collections import deque

    for p in qkv_pieces(0):
        p()

    for j in range(NJ):
        qt = qt2[:, j % 2]
        fill = deque()
        if j + 1 < NJ:
            fill.extend(qkv_pieces(j + 1))
        if j > 0:
            fill.extend(proj_pieces(j - 1, prev01, prev2))

        # ---- streaming attention for this tq chunk ----
        nblk = 4 * j + 4
        npair = nblk // 2
        for h in range(HPC):
            # scores + exp for all key-block pairs of this (j, h)
            pts, cols = [], {}
            for ip in range(npair):
                i0, i1 = 2 * ip, 2 * ip + 1
                # column trim offsets: block i only contributes to tq columns
                # >= 128*(i-4j) within this chunk (the rest is fully masked)
                offs = [max(0, 128 * (i - 4 * j)) for i in (i0, i1)]
                ns = [TQ - o for o in offs]
                starts = [0, ns[0]]
                w = ns[0] + ns[1]
                pss = ps_s.tile([128, 2 * TQ], F32, tag="pss")
                for n, i in enumerate((i0, i1)):
                    cols[i] = (ip, starts[n] - offs[n])
                    nc.tensor.matmul(
                        pss[:, starts[n]:starts[n] + ns[n]],
                        lhsT=KT[h][:, i * 128:(i + 1) * 128],
                        rhs=qt[:, h, offs[n]:TQ],
                        start=True,
                        stop=True,
                    )
                pt = ptp.tile([128, 2 * TQ], BF16, tag="pt")
                pts.append(pt)
                nc.scalar.activation(
                    out=pt[:, :w], in_=pss[:, :w], func=EXP, scale=SCALE
                )
                rp = ip - 2 * j
                if rp >= 0:  # diagonal pair: packed masks for both blocks
                    moff = 0 if rp == 0 else 896
                    nc.vector.tensor_mul(
                        pt[:, :w], pt[:, :w], mask_sb[:, moff:moff + w]
                    )
                if DEBUG and j == 0 and h == 0 and ip == 0:
                    nc.sync.dma_start(t["dbg_pt"], pt[:, 0:896])
                if fill:
                    fill.popleft()()
            # flipped P@V, one accumulation group (= one PSUM bank) per
            # 128-query block qb: stationary = P^T slice, moving = V
            rc = rcp.tile([128, 4], F32, tag="rc")
            on = onp.tile([128, 4, D], BF16, tag="on")
            for qb in range(4):
                pso = ps_o.tile([128, 512], F32, tag="pso")
                last = 4 * j + qb
                for i in range(last + 1):
                    ip, base = cols[i]
                    nc.tensor.matmul(
                        pso[:, 0:D + 1],
                        lhsT=pts[ip][:, base + qb * 128:base + qb * 128 + 128],
                        rhs=Vp[h][:, i, :],
                        start=(i == 0),
                        stop=(i == last),
                    )
                # normalize: O rows / per-partition denominator (column D)
                nc.vector.reciprocal(rc[:, qb:qb + 1], pso[:, D:D + 1])
                nc.vector.tensor_scalar_mul(
                    on[:, qb, :], pso[:, 0:D], rc[:, qb:qb + 1]
                )
                if fill:
                    fill.popleft()()
            if DEBUG and j == 0 and h == 0:
                nc.sync.dma_start(t["dbg_rc"], rc[:])
            # transpose O back to [d, q] for the projection
            ptTt = ps_misc.tile([128, TQ], F32, tag="misc", name="ptT")
            ptT = ptTt.bitcast(BF16)[0:64, 0:512].rearrange(
                "p (qb c) -> p qb c", c=128
            )
            for qb in range(4):
                nc.tensor.transpose(ptT[:, qb, :], on[:, qb, :], id_sb[:])
            if h < 2:
                if h == 0:
                    ot01 = otp.tile([128, 4, 128], BF16, tag="ot01")
                nc.vector.tensor_copy(out=ot01[h * 64:(h + 1) * 64], in_=ptT[:])
            else:
                ot2 = otp.tile([64, 4, 128], BF16, tag="ot2")
                nc.vector.tensor_copy(out=ot2[:], in_=ptT[:])
            if fill:
                fill.popleft()()
        if DEBUG and j == 0:
            nc.sync.dma_start(
                t["dbg_ot"], ot01[:].rearrange("p b d -> p (b d)")
            )
        while fill:
            fill.popleft()()
        prev01, prev2 = ot01, ot2
    for p in proj_pieces(NJ - 1, prev01, prev2):
        p()


def build_nc():
    nc = bacc.Bacc("TRN2", target_bir_lowering=False, debug=False)
    t = {}
    t["xT"] = nc.dram_tensor("xT", [C, T], BF16, kind="ExternalInput").ap()
    t["wqkv"] = nc.dram_tensor("wqkv", [C, NQKV], BF16, kind="ExternalInput").ap()
    t["bqkv"] = nc.dram_tensor("bqkv", [5, 128], F32, kind="ExternalInput").ap()
    t["wproj01"] = nc.dram_tensor("wproj01", [128, C], BF16, kind="ExternalInput").ap()
    t["wproj2"] = nc.dram_tensor("wproj2", [64, C], BF16, kind="ExternalInput").ap()
    t["ident"] = nc.dram_tensor("ident", [128, 128], BF16, kind="ExternalInput").ap()
    t["masks"] = nc.dram_tensor("masks", [128, 1280], BF16, kind="ExternalInput").ap()
    t["outT"] = nc.dram_tensor("outT", [C, T], F32, kind="ExternalOutput").ap()
    if DEBUG:
        for nm, shp, dt in [
            ("dbg_qt", [128, HPC * TQ], BF16),
            ("dbg_kt", [128, TQ], BF16),
            ("dbg_vp", [128, 4 * (D + 1)], BF16),
            ("dbg_pt", [128, 896], BF16),
            ("dbg_on", [128, 4 * D], BF16),
            ("dbg_rc", [128, 4], F32),
            ("dbg_ot", [128, 4 * 128], BF16),
        ]:
            t[nm] = nc.dram_tensor(nm, shp, dt, kind="ExternalOutput").ap()
    with tile.TileContext(nc) as tc:
        _mhsa_body(tc, t)
    nc.compile()
    return nc


def make_in_maps(x, W_qkv, b_qkv, W_proj):
    """Shard the full inputs into one input map per core."""
    import ml_dtypes

    x = np.asarray(x, dtype=np.float32)
    W_qkv = np.asarray(W_qkv, dtype=np.float32)
    b_qkv = np.asarray(b_qkv, dtype=np.float32)
    W_proj = np.asarray(W_proj, dtype=np.float32)

    ident = np.eye(128, dtype=ml_dtypes.bfloat16)
    q_idx = np.arange(TQ)
    p_idx = np.arange(128)
    m4 = np.zeros((4, 128, TQ), dtype=np.float32)
    for r in range(4):
        m4[r] = (p_idx[:, None] <= (q_idx[None, :] - 128 * r)).astype(np.float32)
    masks = np.concatenate(
        [m4[0], m4[1][:, 128:], m4[2][:, 256:], m4[3][:, 384:]], axis=1
    ).astype(ml_dtypes.bfloat16)  # [128, 512+384+256+128 = 1280]

    in_maps = []
    for c in range(NCORES):
        b = c // GPB
        g = c % GPB
        heads = [HPC * g + h for h in range(HPC)]

        wg = np.zeros((C, NQKV), dtype=np.float32)
        bg = np.zeros((5, 128), dtype=np.float32)
        qty_off = {"Q": 0, "K": C, "V": 2 * C}
        for m, ents in enumerate(CHUNKS):
            for (qty, h, base) in ents:
                src = qty_off[qty] + heads[h] * D
                wg[:, m * 128 + base:m * 128 + base + D] = W_qkv[:, src:src + D]
                bg[m, base:base + D] = b_qkv[src:src + D]

        wp01 = np.concatenate(
            [
                W_proj[heads[0] * D:(heads[0] + 1) * D, :],
                W_proj[heads[1] * D:(heads[1] + 1) * D, :],
            ],
            axis=0,
        ).astype(ml_dtypes.bfloat16)
        wp2 = W_proj[heads[2] * D:(heads[2] + 1) * D, :].astype(ml_dtypes.bfloat16)

        in_maps.append({
            "xT": np.ascontiguousarray(x[b].T).astype(ml_dtypes.bfloat16),
            "wqkv": wg.astype(ml_dtypes.bfloat16),
            "bqkv": bg,
            "wproj01": wp01,
            "wproj2": wp2,
            "ident": ident,
            "masks": masks,
        })
    return in_maps


def run_cores(inputs, trace=False, **kw):
    nc = build_nc()
    in_maps = make_in_maps(
        inputs["x"], inputs["W_qkv"], inputs["b_qkv"], inputs["W_proj"]
    )
    res = run_bass_kernel_spmd(nc, in_maps, list(range(NCORES)), trace=trace, **kw)
    return res


def gather(results, b_proj):
    out = np.zeros((B, T, C), dtype=np.float32)
    for c in range(NCORES):
        out[c // GPB] += results[c]["outT"].T
    out += np.asarray(b_proj, dtype=np.float32)
    return out


def kernel(x, W_qkv, b_qkv, W_proj, b_proj):
    res = run_cores(
        {"x": x, "W_qkv": W_qkv, "b_qkv": b_qkv, "W_proj": W_proj}
    )
    return gather(res.results, b_proj)


# revision 21
# speedup vs baseline: 1.2749x; 1.0554x over previous
"""Multi-head causal self-attention (B=2, T=4096, C=768, H=12, D=64) on 8 NeuronCores.

Sharding: core c handles batch b = c // 4 and a group of 3 heads (c % 4).
Each core runs a fused flash-attention pipeline per 512-column tq chunk:
QKV projection -> V transpose -> streaming softmax(QK^T)V -> output projection,
producing a partial (pre-bias) out.T [768, 4096]. The host sums the 4 partials
per batch and adds the projection bias.

v2 layout: the P@V matmul is flipped to produce O natural [q, d] (moving dim =
d+1 = 65 instead of 512), which makes the softmax denominator a per-partition
scalar: reciprocal [128,1] + per-partition multiply, no cross-partition
broadcast. O is then transposed back via PE for the output projection, whose
per-head weights are stacked 2+1 on the contraction dim. exp runs without
max-subtraction (scores are O(+-6)); the denominator comes free from a
ones-column appended to V.
"""

from contextlib import ExitStack

import numpy as np

import concourse.bass as bass
import concourse.tile as tile
from concourse import bacc
from concourse import mybir
from concourse._compat import with_exitstack
from concourse.bass_utils import run_bass_kernel_spmd

F32 = mybir.dt.float32
F32R = mybir.dt.float32r
BF16 = mybir.dt.bfloat16
EXP = mybir.ActivationFunctionType.Exp

B, T, C = 2, 4096, 768
H, D = 12, 64
NCORES = 8
HPC = 3           # heads per core
GPB = NCORES // B  # head-group cores per batch (4)
TQ = 512          # tq chunk width
NJ = T // TQ      # 8
TKB = 128         # tk block
NB = T // TKB     # 32
KC = C // 128     # 6 contraction chunks for the QKV projection
SCALE = 1.0 / np.sqrt(D)

# Layout of the per-core QKV weight columns: 5 chunks of 128 (last half-used).
# Each entry is (quantity, local head, partition base within the chunk).
# Chosen so Q and K of the same head land on the same partition half (their
# zero-padded halves line up in the 128-deep score contraction).
CHUNKS = [
    [("Q", 0, 0), ("Q", 1, 64)],
    [("Q", 2, 0), ("V", 0, 64)],
    [("K", 0, 0), ("K", 1, 64)],
    [("K", 2, 0), ("V", 1, 64)],
    [("V", 2, 0)],
]
NQKV = 4 * 128 + 64  # 576 columns of per-core qkv weights

QK_BASE = {0: 0, 1: 64, 2: 0}   # partition base of Q/K data per local head

DEBUG = False  # add j=0 intermediate dumps for bisection


def _proj(nc, ps_p, stp, wp01_sb, wp2_sb, outT_r, ot01, ot2, j):
    jsl = slice(j * TQ, (j + 1) * TQ)
    for m in range(KC):
        ps3 = ps_p.tile([128, TQ], F32, tag="misc", name="ps3")
        nc.tensor.matmul(
            ps3[:],
            lhsT=wp01_sb[:, m * 128:(m + 1) * 128],
            rhs=ot01[:],
            start=True,
            stop=False,
        )
        nc.tensor.matmul(
            ps3[:],
            lhsT=wp2_sb[:, m * 128:(m + 1) * 128],
            rhs=ot2[:],
            start=False,
            stop=True,
        )
        st = stp.tile([128, TQ], F32, tag="st", name="st")
        nc.vector.tensor_copy(st[:], ps3[:])
        nc.sync.dma_start(outT_r[:, m, jsl], st[:])


@with_exitstack
def _mhsa_body(ctx: ExitStack, tc: tile.TileContext, t):
    nc = tc.nc
    xT_r = t["xT"].rearrange("(kc p) t -> p kc t", p=128)
    outT_r = t["outT"].rearrange("(mo p) t -> p mo t", p=128)

    const = ctx.enter_context(tc.tile_pool(name="const", bufs=1))
    persist = ctx.enter_context(tc.tile_pool(name="persist", bufs=1))
    xpool = ctx.enter_context(tc.tile_pool(name="xpool", bufs=2))
    vstp = ctx.enter_context(tc.tile_pool(name="vstp", bufs=2))
    ptp = ctx.enter_context(tc.tile_pool(name="ptp", bufs=18))
    onp = ctx.enter_context(tc.tile_pool(name="onp", bufs=2))
    rcp = ctx.enter_context(tc.tile_pool(name="rcp", bufs=4))
    otp = ctx.enter_context(tc.tile_pool(name="otp", bufs=2))
    stp = ctx.enter_context(tc.tile_pool(name="stp", bufs=3))

    ps_s = ctx.enter_context(tc.tile_pool(name="ps_s", bufs=2, space="PSUM"))
    ps_o = ctx.enter_context(tc.tile_pool(name="ps_o", bufs=2, space="PSUM"))
    ps_misc = ctx.enter_context(tc.tile_pool(name="ps_misc", bufs=2, space="PSUM"))

    wq_sb = const.tile([128, KC, NQKV], BF16)
    nc.sync.dma_start(wq_sb[:], t["wqkv"].rearrange("(kc p) m -> p kc m", p=128))
    bias_sb = const.tile([128, 5], F32)
    nc.sync.dma_start(bias_sb[:], t["bqkv"].rearrange("m p -> p m"))
    wp01_sb = const.tile([128, C], BF16)
    nc.sync.dma_start(wp01_sb[:], t["wproj01"])
    wp2_sb = const.tile([64, C], BF16)
    nc.sync.dma_start(wp2_sb[:], t["wproj2"])
    id_sb = const.tile([128, 128], BF16)
    nc.sync.dma_start(id_sb[:], t["ident"])
    mask_sb = const.tile([128, 1280], BF16)
    nc.sync.dma_start(mask_sb[:], t["masks"])

    KT = [persist.tile([128, T], BF16, tag=f"KT{h}", name=f"KT{h}") for h in range(HPC)]
    Vp = [
        persist.tile([128, NB, D + 1], BF16, tag=f"Vp{h}", name=f"Vp{h}")
        for h in range(HPC)
    ]
    # persistent double-buffered qt: pads zeroed once, never rewritten
    qt2 = persist.tile([128, 2, HPC, TQ], BF16, tag="qt2", name="qt2")

    for h in range(HPC):
        pad_lo = 64 - QK_BASE[h]  # 64 if data at 0, 0 if data at 64
        nc.vector.memset(KT[h][pad_lo:pad_lo + 64, :], 0.0)
        nc.vector.memset(Vp[h][:, :, D:D + 1], 1.0)
        for db in range(2):
            nc.vector.memset(qt2[pad_lo:pad_lo + 64, db, h, :], 0.0)

    def qkv_pieces(j):
        """Closures emitting QKV slabs + V transposes for chunk j; xt DMA now."""
        jsl = slice(j * TQ, (j + 1) * TQ)
        qt = qt2[:, j % 2]
        xt = xpool.tile([128, KC, TQ], BF16, tag="xt")
        nc.sync.dma_start(xt[:], xT_r[:, :, jsl])
        vst = {}
        pieces = []

        slab_ps = {}

        def slab_kc(m, ents, mw, kc):
            if kc == 0:
                slab_ps[m] = ps_misc.tile([128, TQ], F32, tag="misc", name="psq")
            ps = slab_ps[m]
            nc.tensor.matmul(
                ps[:mw],
                lhsT=wq_sb[:, kc, m * 128:m * 128 + mw],
                rhs=xt[:, kc, :],
                start=(kc == 0),
                stop=(kc == KC - 1),
            )
            if kc < KC - 1:
                return
            for (qty, h, base) in ents:
                if qty == "V":
                    vt = vstp.tile([64, TQ], BF16, tag=f"vst{h}")
                    vst[h] = vt
                    dst = vt[:]
                elif qty == "K":
                    dst = KT[h][base:base + 64, jsl]
                else:
                    dst = qt[base:base + 64, h, :]
                nc.vector.tensor_scalar_add(
                    dst, ps[base:base + 64, :], bias_sb[base:base + 64, m:m + 1]
                )

        for m in range(5):
            ents = CHUNKS[m]
            mw = 128 if len(ents) == 2 else 64
            for kc in range(KC):
                pieces.append(
                    lambda m=m, ents=ents, mw=mw, kc=kc: slab_kc(m, ents, mw, kc)
                )

        def vtr(h):
            # 4 packed transposes (atomic start+stop groups) + one copy
            vt = vst[h]
            pst = ps_misc.tile([128, TQ], F32, tag="misc", name="pst")
            pstv = pst.bitcast(BF16)
            for sl in range(4):
                nc.tensor.transpose(
                    pstv[:, sl * 64:sl * 64 + 64],
                    vt[:, sl * 128:(sl + 1) * 128],
                    id_sb[0:64, 0:64],
                )
            nc.vector.tensor_copy(
                out=Vp[h][:, 4 * j:4 * j + 4, 0:D],
                in_=pstv[:, 0:256].rearrange("p (s d) -> p s d", d=D),
            )

        for h in range(HPC):
            pieces.append(lambda h=h: vtr(h))
        return pieces

    def proj_pieces(j, ot01, ot2):
        pieces = []

        def pm(m):
            jsl = slice(j * TQ, (j + 1) * TQ)
            ps3 = ps_misc.tile([128, TQ], F32, tag="misc", name="ps3")
            nc.tensor.matmul(
                ps3[:], lhsT=wp01_sb[:, m * 128:(m + 1) * 128], rhs=ot01[:],
                start=True, stop=False,
            )
            nc.tensor.matmul(
                ps3[:], lhsT=wp2_sb[:, m * 128:(m + 1) * 128], rhs=ot2[:],
                start=False, stop=True,
            )
            st = stp.tile([128, TQ], F32, tag="st", name="st")
            nc.vector.tensor_copy(st[:], ps3[:])
            nc.sync.dma_start(outT_r[:, m, jsl], st[:])

        for m in range(KC):
            pieces.append(lambda m=m: pm(m))
        return pieces

    from collections import deque

    for p in qkv_pieces(0):
        p()

    for j in range(NJ):
        qt = qt2[:, j % 2]
        fill = deque()
        if j + 1 < NJ:
            fill.extend(qkv_pieces(j + 1))
        if j > 0:
            fill.extend(proj_pieces(j - 1, prev01, prev2))
        nslots = 3 * (2 * j + 2) + 12 + 3
        nfill = len(fill)
        drained = 0
        slot_i = 0

        def drain():
            nonlocal slot_i, drained
            slot_i += 1
            want = (nfill * slot_i) // nslots
            while drained < want and fill:
                fill.popleft()()
                drained += 1

        # ---- streaming attention for this tq chunk ----
        nblk = 4 * j + 4
        npair = nblk // 2
        for h in range(HPC):
            # scores + exp for all key-block pairs of this (j, h)
            pts, cols = [], {}
            for ip in range(npair):
                i0, i1 = 2 * ip, 2 * ip + 1
                # column trim offsets: block i only contributes to tq columns
                # >= 128*(i-4j) within this chunk (the rest is fully masked)
                offs = [max(0, 128 * (i - 4 * j)) for i in (i0, i1)]
                ns = [TQ - o for o in offs]
                starts = [0, ns[0]]
                w = ns[0] + ns[1]
                pss = ps_s.tile([128, 2 * TQ], F32, tag="pss")
                for n, i in enumerate((i0, i1)):
                    cols[i] = (ip, starts[n] - offs[n])
                    nc.tensor.matmul(
                        pss[:, starts[n]:starts[n] + ns[n]],
                        lhsT=KT[h][:, i * 128:(i + 1) * 128],
                        rhs=qt[:, h, offs[n]:TQ],
                        start=True,
                        stop=True,
                    )
                pt = ptp.tile([128, 2 * TQ], BF16, tag="pt")
                pts.append(pt)
                nc.scalar.activation(
                    out=pt[:, :w], in_=pss[:, :w], func=EXP, scale=SCALE
                )
                rp = ip - 2 * j
                if rp >= 0:  # diagonal pair: packed masks for both blocks
                    moff = 0 if rp == 0 else 896
                    nc.vector.tensor_mul(
                        pt[:, :w], pt[:, :w], mask_sb[:, moff:moff + w]
                    )
                if DEBUG and j == 0 and h == 0 and ip == 0:
                    nc.sync.dma_start(t["dbg_pt"], pt[:, 0:896])
                drain()
            # flipped P@V, one accumulation group (= one PSUM bank) per
            # 128-query block qb: stationary = P^T slice, moving = V
            rc = rcp.tile([128, 4], F32, tag="rc")
            on = onp.tile([128, 4, D], BF16, tag="on")
            for qb in range(4):
                pso = ps_o.tile([128, 512], F32, tag="pso")
                last = 4 * j + qb
                for i in range(last + 1):
                    ip, base = cols[i]
                    nc.tensor.matmul(
                        pso[:, 0:D + 1],
                        lhsT=pts[ip][:, base + qb * 128:base + qb * 128 + 128],
                        rhs=Vp[h][:, i, :],
                        start=(i == 0),
                        stop=(i == last),
                    )
                # normalize: O rows / per-partition denominator (column D)
                nc.vector.reciprocal(rc[:, qb:qb + 1], pso[:, D:D + 1])
                nc.vector.tensor_scalar_mul(
                    on[:, qb, :], pso[:, 0:D], rc[:, qb:qb + 1]
                )
                drain()
            if DEBUG and j == 0 and h == 0:
                nc.sync.dma_start(t["dbg_rc"], rc[:])
            # transpose O back to [d, q] for the projection
            ptTt = ps_misc.tile([128, TQ], F32, tag="misc", name="ptT")
            ptT = ptTt.bitcast(BF16)[0:64, 0:512].rearrange(
                "p (qb c) -> p qb c", c=128
            )
            for qb in range(4):
                nc.tensor.transpose(ptT[:, qb, :], on[:, qb, :], id_sb[:])
            if h < 2:
                if h == 0:
                    ot01 = otp.tile([128, 4, 128], BF16, tag="ot01")
                nc.vector.tensor_copy(out=ot01[h * 64:(h + 1) * 64], in_=ptT[:])
            else:
                ot2 = otp.tile([64, 4, 128], BF16, tag="ot2")
                nc.vector.tensor_copy(out=ot2[:], in_=ptT[:])
            drain()
        if DEBUG and j == 0:
            nc.sync.dma_start(
                t["dbg_ot"], ot01[:].rearrange("p b d -> p (b d)")
            )
        while fill:
            fill.popleft()()
        prev01, prev2 = ot01, ot2
    for p in proj_pieces(NJ - 1, prev01, prev2):
        p()


def build_nc():
    nc = bacc.Bacc("TRN2", target_bir_lowering=False, debug=False)
    t = {}
    t["xT"] = nc.dram_tensor("xT", [C, T], BF16, kind="ExternalInput").ap()
    t["wqkv"] = nc.dram_tensor("wqkv", [C, NQKV], BF16, kind="ExternalInput").ap()
    t["bqkv"] = nc.dram_tensor("bqkv", [5, 128], F32, kind="ExternalInput").ap()
    t["wproj01"] = nc.dram_tensor("wproj01", [128, C], BF16, kind="ExternalInput").ap()
    t["wproj2"] = nc.dram_tensor("wproj2", [64, C], BF16, kind="ExternalInput").ap()
    t["ident"] = nc.dram_tensor("ident", [128, 128], BF16, kind="ExternalInput").ap()
    t["masks"] = nc.dram_tensor("masks", [128, 1280], BF16, kind="ExternalInput").ap()
    t["outT"] = nc.dram_tensor("outT", [C, T], F32, kind="ExternalOutput").ap()
    if DEBUG:
        for nm, shp, dt in [
            ("dbg_qt", [128, HPC * TQ], BF16),
            ("dbg_kt", [128, TQ], BF16),
            ("dbg_vp", [128, 4 * (D + 1)], BF16),
            ("dbg_pt", [128, 896], BF16),
            ("dbg_on", [128, 4 * D], BF16),
            ("dbg_rc", [128, 4], F32),
            ("dbg_ot", [128, 4 * 128], BF16),
        ]:
            t[nm] = nc.dram_tensor(nm, shp, dt, kind="ExternalOutput").ap()
    with tile.TileContext(nc) as tc:
        _mhsa_body(tc, t)
    nc.compile()
    return nc


def make_in_maps(x, W_qkv, b_qkv, W_proj):
    """Shard the full inputs into one input map per core."""
    import ml_dtypes

    x = np.asarray(x, dtype=np.float32)
    W_qkv = np.asarray(W_qkv, dtype=np.float32)
    b_qkv = np.asarray(b_qkv, dtype=np.float32)
    W_proj = np.asarray(W_proj, dtype=np.float32)

    ident = np.eye(128, dtype=ml_dtypes.bfloat16)
    q_idx = np.arange(TQ)
    p_idx = np.arange(128)
    m4 = np.zeros((4, 128, TQ), dtype=np.float32)
    for r in range(4):
        m4[r] = (p_idx[:, None] <= (q_idx[None, :] - 128 * r)).astype(np.float32)
    masks = np.concatenate(
        [m4[0], m4[1][:, 128:], m4[2][:, 256:], m4[3][:, 384:]], axis=1
    ).astype(ml_dtypes.bfloat16)  # [128, 512+384+256+128 = 1280]

    in_maps = []
    for c in range(NCORES):
        b = c // GPB
        g = c % GPB
        heads = [HPC * g + h for h in range(HPC)]

        wg = np.zeros((C, NQKV), dtype=np.float32)
        bg = np.zeros((5, 128), dtype=np.float32)
        qty_off = {"Q": 0, "K": C, "V": 2 * C}
        for m, ents in enumerate(CHUNKS):
            for (qty, h, base) in ents:
                src = qty_off[qty] + heads[h] * D
                wg[:, m * 128 + base:m * 128 + base + D] = W_qkv[:, src:src + D]
                bg[m, base:base + D] = b_qkv[src:src + D]

        wp01 = np.concatenate(
            [
                W_proj[heads[0] * D:(heads[0] + 1) * D, :],
                W_proj[heads[1] * D:(heads[1] + 1) * D, :],
            ],
            axis=0,
        ).astype(ml_dtypes.bfloat16)
        wp2 = W_proj[heads[2] * D:(heads[2] + 1) * D, :].astype(ml_dtypes.bfloat16)

        in_maps.append({
            "xT": np.ascontiguousarray(x[b].T).astype(ml_dtypes.bfloat16),
            "wqkv": wg.astype(ml_dtypes.bfloat16),
            "bqkv": bg,
            "wproj01": wp01,
            "wproj2": wp2,
            "ident": ident,
            "masks": masks,
        })
    return in_maps


def run_cores(inputs, trace=False, **kw):
    nc = build_nc()
    in_maps = make_in_maps(
        inputs["x"], inputs["W_qkv"], inputs["b_qkv"], inputs["W_proj"]
    )
    res = run_bass_kernel_spmd(nc, in_maps, list(range(NCORES)), trace=trace, **kw)
    return res


def gather(results, b_proj):
    out = np.zeros((B, T, C), dtype=np.float32)
    for c in range(NCORES):
        out[c // GPB] += results[c]["outT"].T
    out += np.asarray(b_proj, dtype=np.float32)
    return out


def kernel(x, W_qkv, b_qkv, W_proj, b_proj):
    res = run_cores(
        {"x": x, "W_qkv": W_qkv, "b_qkv": b_qkv, "W_proj": W_proj}
    )
    return gather(res.results, b_proj)
